# revision 1
# baseline (speedup 1.0000x reference)
"""Distributed Trainium2 Bass kernel for nn_AttentionLayer_25993142075512.

Sharding: 8 cores = 2 batches x 4 head-groups (4 heads each). Each core
computes its batch's q/k/v projections for its 4 heads, causal attention,
and a partial output projection o @ Wo[head_rows]. Host sums the 4
partials per batch and adds bo. No on-device collectives.

Layout tricks:
  - All activations enter transposed (d on partitions): qT/kT/vT come out
    of weight-stationary matmuls directly.
  - RoPE: head dims are permuted host-side (evens @0:16, pass @16:32,
    odds @32:48, pass @48:64) for BOTH q and k (dot products invariant),
    so rotate_every_two becomes contiguous 16-partition block ops on DVE
    at legal partition bases (0/32/64/96).
  - Scores are computed transposed (kj on partitions) so softmaxed probs
    feed the o-matmul as the moving operand with kj as contraction dim.
  - Softmax denominator = ones column appended to v (row 64 of oT).
  - exp(0.125*s + causal_mask + kv_mask_bias) fused in one ACT op.
  - 1/denom applied to oT via a rank-1 broadcast matmul + DVE multiply.
  - q-blocks are processed in packs of 4 (512 cols) so scores/exp/oT ops
    amortize per-instruction overhead; PSUM accumulation uses one
    has_written group per 2KB zero-region (bank).

Assumes mask_q == 1 (spec fill=ones); mask_kv handled exactly.
"""

import sys, os, types, ctypes, contextlib

sys.path.insert(0, "/opt/trn_rl_repo")

import numpy as np
import ml_dtypes


def _install_axon_hooks():
    so = "/opt/axon/libaxon_pjrt.so"

    def _hook_factory(so_path):
        if not os.path.exists(so_path):
            return None
        lib = ctypes.CDLL(so_path)
        if not hasattr(lib, "axon_start_nrt_profile"):
            return None
        lib.axon_start_nrt_profile.argtypes = [
            ctypes.POINTER(ctypes.c_int64),
            ctypes.c_size_t,
        ]
        lib.axon_start_nrt_profile.restype = ctypes.c_int64
        lib.axon_stop_nrt_profile.argtypes = [ctypes.c_char_p]
        lib.axon_stop_nrt_profile.restype = ctypes.c_int64

        @contextlib.contextmanager
        def _hook(output_dir, device_ids):
            import jax

            jax.devices()
            if device_ids:
                ids = (ctypes.c_int64 * len(device_ids))(*device_ids)
                rc = lib.axon_start_nrt_profile(ids, len(device_ids))
            else:
                rc = lib.axon_start_nrt_profile(None, 0)
            if rc != 0:
                raise RuntimeError(f"axon_start_nrt_profile rc={rc}")
            try:
                yield
            finally:
                n = lib.axon_stop_nrt_profile(str(output_dir).encode())
                if n < 0:
                    raise RuntimeError(f"axon_stop_nrt_profile rc={n}")

        return _hook

    try:
        import antenv

        if "antenv.axon_hooks" not in sys.modules:
            hook = _hook_factory(so)
            mod = types.ModuleType("antenv.axon_hooks")
            mod.get_axon_ntff_profile_hook = lambda: hook
            mod.set_axon_ntff_profile_hook = lambda h: None
            antenv.axon_hooks = mod
            sys.modules["antenv.axon_hooks"] = mod
    except ImportError:
        pass
    from concourse import bass_utils

    bass_utils.upload_artifacts = lambda tmpdir: tmpdir


_install_axon_hooks()

from concourse import bass, bacc, tile, mybir  # noqa: E402

BF16 = mybir.dt.bfloat16
F32 = mybir.dt.float32
NPBF16 = ml_dtypes.bfloat16

B, N, DQ, DKV, H, DH, DOUT = 2, 2048, 1024, 1024, 16, 64, 1024
ROT = DH // 2  # 32
INF = 1.0e6
HPC = 4  # heads per core
NB = N // 128  # 16 q/k blocks
NG = NB // 4  # 4 q-block groups (packs of 4)
NQ4 = 4  # n quarters for projections
NQW = N // NQ4  # 512
VS = 72  # v' tile stride (64 v cols + ones col + pad)


def _head_perm():
    """Permute one head's 64 dims so RoPE even/odd blocks start at partition
    offsets 0 and 32: [evens(0,2..30), pass 32:48, odds(1,3..31), pass 48:64]."""
    ev = np.arange(0, ROT, 2)
    od = np.arange(1, ROT, 2)
    return np.concatenate([ev, np.arange(32, 48), od, np.arange(48, 64)])


def build_nc():
    _KP = int(os.environ.get("BASS_KERNEL_ABLATE", "5"))
    nc = bacc.Bacc(None, target_bir_lowering=False)

    sqt = nc.declare_dram_parameter("sqt", [DQ, N], BF16, isOutput=False)
    skvt = nc.declare_dram_parameter("skvt", [DKV, N], BF16, isOutput=False)
    wq = nc.declare_dram_parameter("wq", [8, 128, HPC * DH], BF16, isOutput=False)
    wkv = nc.declare_dram_parameter("wkv", [8, 128, HPC * 2 * DH], BF16, isOutput=False)
    wo = nc.declare_dram_parameter("wo", [2, 128, DOUT], BF16, isOutput=False)
    bq = nc.declare_dram_parameter("bq", [2, 128, 1], F32, isOutput=False)
    bkv = nc.declare_dram_parameter("bkv", [HPC, 128, 1], F32, isOutput=False)
    cost_d = nc.declare_dram_parameter("cost", [128, N], BF16, isOutput=False)
    sint_d = nc.declare_dram_parameter("sint", [128, N], BF16, isOutput=False)
    mtile_d = nc.declare_dram_parameter("mtile", [128, 128], BF16, isOutput=False)
    ident_d = nc.declare_dram_parameter("ident", [128, 128], BF16, isOutput=False)
    bmask_d = nc.declare_dram_parameter("bmask", [NB, 128, 1], F32, isOutput=False)
    out_ext = nc.declare_dram_parameter("out", [N, DOUT], BF16, isOutput=True)

    AF = mybir.ActivationFunctionType
    ALU = mybir.AluOpType

    with tile.TileContext(nc) as tc:
        with (
            tc.tile_pool(name="const", bufs=1) as cpool,
            tc.tile_pool(name="big", bufs=1) as bigpool,
            tc.tile_pool(name="stream", bufs=4) as spool,
            tc.tile_pool(name="ptile", bufs=8) as ppool,
            tc.tile_pool(name="small", bufs=8) as smallpool,
            tc.tile_pool(name="outsb", bufs=4) as outsb_pool,
        ):
            # ---- constants to SBUF ----
            wq_sb = []
            wkv_sb = []
            for c in range(8):
                t = cpool.tile([128, HPC * DH], BF16, tag=f"wq{c}", name=f"wq{c}")
                nc.sync.dma_start(t[:], wq[c])
                wq_sb.append(t)
                t2 = cpool.tile([128, HPC * 2 * DH], BF16, tag=f"wkv{c}", name=f"wkv{c}")
                nc.sync.dma_start(t2[:], wkv[c])
                wkv_sb.append(t2)
            bq_sb = cpool.tile([128, 2], F32, tag="bq", name="bq")
            for m in range(2):
                nc.sync.dma_start(bq_sb[:, m : m + 1], bq[m])
            bkv_sb = cpool.tile([128, HPC], F32, tag="bkv", name="bkv")
            for h in range(HPC):
                nc.sync.dma_start(bkv_sb[:, h : h + 1], bkv[h])
            # later-phase constants: tiles declared here, DMAs issued after the
            # projection stream so the first matmuls aren't queued behind them
            wo_sb = [cpool.tile([128, DOUT], BF16, tag=f"wo{pr}", name=f"wo{pr}") for pr in range(2)]
            cost = cpool.tile([128, N], BF16, tag="cost", name="cost")
            sint = cpool.tile([128, N], BF16, tag="sint", name="sint")
            mtile = cpool.tile([128, 128], BF16, tag="mtile", name="mtile")
            ident = cpool.tile([128, 128], BF16, tag="ident", name="ident")
            bmask = cpool.tile([128, NB], F32, tag="bmask", name="bmask")
            ones1 = cpool.tile([1, 64], BF16, tag="ones1", name="ones1")

            def _late_const_dmas():
                nc.sync.dma_start(cost[:], cost_d[:])
                nc.sync.dma_start(sint[:], sint_d[:])
                nc.sync.dma_start(mtile[:], mtile_d[:])
                nc.sync.dma_start(ident[:], ident_d[:])
                for kb in range(NB):
                    nc.sync.dma_start(bmask[:, kb : kb + 1], bmask_d[kb])
                for pr in range(2):
                    nc.sync.dma_start(wo_sb[pr][:], wo[pr])
                nc.vector.memset(ones1[:], 1.0)

            # ---- persistent activations ----
            qT = [bigpool.tile([128, N], BF16, tag=f"qT{i}", name=f"qT{i}") for i in range(2)]
            # kvT per head [128, N]: even head: kT rows 0:64, vT rows 64:128;
            # odd head: vT rows 0:64, kT rows 64:128 (parity-matched bases).
            kvT = [bigpool.tile([128, N], BF16, tag=f"kvT{h}", name=f"kvT{h}") for h in range(HPC)]
            # v' group tiles per (head, group): [128, 4, VS]; [:, j, 0:64] = v
            # for kb=4g+j, [:, j, 64] = ones (whole tile memset to 1 first).
            vg = [
                [bigpool.tile([128, 4, VS], BF16, tag=f"vg{h}_{g}", name=f"vg{h}_{g}") for g in range(NG)]
                for h in range(HPC)
            ]
            # normalized oT groups per (pair, group): [128, 512] bf16
            oTs = [
                [bigpool.tile([128, 512], BF16, tag=f"oTs{pr}_{g}", name=f"oTs{pr}_{g}") for g in range(NG)]
                for pr in range(2)
            ]

            def rope_block(dst, r0, c0=0, cw=N):
                """Rotary in-place: evens at dst[r0:r0+16], odds at
                dst[r0+32:r0+48], columns [c0, c0+cw)."""
                cs = slice(c0, c0 + cw)
                e = slice(r0, r0 + 16)
                o = slice(r0 + 32, r0 + 48)
                cE, sE = cost[e, cs], sint[e, cs]
                cO, sO = cost[o, cs], sint[o, cs]
                # plain tensor_tensor ops run in DVE 2x mode for bf16 (stt
                # has no fast mode). Sign of sin is baked into the host table:
                # odd-block rows hold -sin, even-block rows hold +sin.
                t1 = smallpool.tile([16, cw], BF16, tag="ropetmp1", name="ropetmp1", bufs=2)
                t2 = smallpool.tile([16, cw], BF16, tag="ropetmp2", name="ropetmp2", bufs=2)
                t3 = smallpool.tile([16, cw], BF16, tag="ropetmp3", name="ropetmp3", bufs=2)
                t4 = smallpool.tile([16, cw], BF16, tag="ropetmp4", name="ropetmp4", bufs=2)
                v = nc.vector
                v.tensor_mul(t1[:], dst[e, cs], cE)
                v.tensor_mul(t2[:], dst[o, cs], sO)
                v.tensor_mul(t3[:], dst[o, cs], cO)
                v.tensor_mul(t4[:], dst[e, cs], sE)
                v.tensor_add(dst[e, cs], t1[:], t2[:])
                v.tensor_add(dst[o, cs], t3[:], t4[:])

            # ================= phase 1: projections =================
            with tc.tile_pool(name="projpsum", bufs=1, space=bass.MemorySpace.PSUM) as pj:
                for nhf in range(2 if _KP >= 1 else 0):
                    h0_ = nhf * (N // 2)
                    xqs, xkvs = [], []
                    for c in range(8):
                        xq = spool.tile([128, N // 2], BF16, tag="xq", name="xq", bufs=10)
                        nc.sync.dma_start(xq[:], sqt[c * 128 : (c + 1) * 128, h0_ : h0_ + N // 2])
                        xkvt = spool.tile([128, N // 2], BF16, tag="xkv", name="xkv", bufs=10)
                        nc.sync.dma_start(xkvt[:], skvt[c * 128 : (c + 1) * 128, h0_ : h0_ + N // 2])
                        xqs.append(xq)
                        xkvs.append(xkvt)
                    if nhf == 0:
                        _late_const_dmas()
                    for sub in range(2):
                        nq0 = h0_ + sub * NQW
                        s0 = sub * NQW
                        ps_q = [pj.tile([128, NQW], F32, tag=f"psq{m}", name=f"psq{m}", bufs=2) for m in range(2)]
                        ps_kv = [pj.tile([128, NQW], F32, tag=f"pskv{h}", name=f"pskv{h}") for h in range(HPC)]
                        for c in range(8):
                            st = c == 0
                            sp = c == 7
                            for m in range(2):
                                nc.tensor.matmul(
                                    ps_q[m][:],
                                    wq_sb[c][:, m * 128 : (m + 1) * 128],
                                    xqs[c][:, s0 : s0 + NQW],
                                    start=st,
                                    stop=sp,
                                )
                            for h in range(HPC):
                                nc.tensor.matmul(
                                    ps_kv[h][:],
                                    wkv_sb[c][:, h * 128 : (h + 1) * 128],
                                    xkvs[c][:, s0 : s0 + NQW],
                                    start=st,
                                    stop=sp,
                                )
                        for m in range(2):
                            nc.scalar.activation(
                                qT[m][:, nq0 : nq0 + NQW],
                                ps_q[m][:],
                                AF.Identity,
                                bias=bq_sb[:, m : m + 1],
                            )
                        for h in range(HPC):
                            nc.scalar.activation(
                                kvT[h][:, nq0 : nq0 + NQW],
                                ps_kv[h][:],
                                AF.Identity,
                                bias=bkv_sb[:, h : h + 1],
                            )
                        if _KP >= 2:
                            for m in range(2):
                                rope_block(qT[m], 0, nq0, NQW)
                                rope_block(qT[m], 64, nq0, NQW)
                            for h in range(HPC):
                                rope_block(kvT[h], (h % 2) * 64, nq0, NQW)


            # ================= phase 2: v' build (transpose vT) =================
            with tc.tile_pool(name="vtpsum", bufs=2, space=bass.MemorySpace.PSUM) as vtp:
                for h in range(HPC if _KP >= 3 else 0):
                    vb = 64 if h % 2 == 0 else 0  # v rows base (host layout)
                    for g in range(NG):
                        nc.vector.memset(vg[h][g][:], 1.0)
                        pk = vtp.tile([128, 256], BF16, tag="vtp", name="vtp")
                        for j in range(4):
                            kb = 4 * g + j
                            nc.tensor.matmul(
                                pk[:, j * 64 : (j + 1) * 64],
                                kvT[h][vb : vb + 64, kb * 128 : (kb + 1) * 128],
                                ident[vb : vb + 64, vb : vb + 64],
                                is_transpose=True,
                                start=(j == 0),
                                stop=(j == 3),
                            )
                        nc.scalar.activation(vg[h][g][:, :, 0:64], pk[:], AF.Copy)

            # ================= phase 3: attention =================
            with (
                tc.tile_pool(name="stpsum", bufs=3, space=bass.MemorySpace.PSUM) as stp,
                tc.tile_pool(name="otpsum", bufs=1, space=bass.MemorySpace.PSUM) as otp,
            ):
                for h in range(HPC if _KP >= 4 else 0):
                    pr, hr = h // 2, (h % 2) * 64
                    kr = (h % 2) * 64  # k rows base (parity-matched to q slice)
                    oT = [otp.tile([65, 512], F32, tag=f"oT{g}", name=f"oT{g}") for g in range(NG)]
                    for kb in range(NB):
                        for g in range(kb // 4, NG):
                            q0 = max(kb, 4 * g)
                            off = (q0 % 4) * 128
                            w = (4 * g + 4 - q0) * 128
                            sTp = stp.tile([128, 512], F32, tag="sT", name="sT")
                            nc.tensor.matmul(
                                sTp[:, off : off + w],
                                kvT[h][kr : kr + 64, kb * 128 : (kb + 1) * 128],
                                qT[pr][hr : hr + 64, q0 * 128 : q0 * 128 + w],
                                start=True,
                                stop=True,
                            )
                            if q0 == kb:  # diagonal block: causal mask on DVE
                                nc.vector.tensor_add(
                                    sTp[:, off : off + 128],
                                    sTp[:, off : off + 128],
                                    mtile[:],
                                )
                            p = ppool.tile([128, 512], BF16, tag="p", name="p")
                            nc.scalar.activation(
                                p[:, off : off + w],
                                sTp[:, off : off + w],
                                AF.Exp,
                                bias=bmask[:, kb : kb + 1],
                                scale=0.125,
                            )
                            # one has_written group per PSUM bank: start zeroes
                            # the whole zero-region once (kb==0 writes the full
                            # 512 span); later partial spans overwrite stale
                            # slices on first touch, then accumulate.
                            nc.tensor.matmul(
                                oT[g][:, off : off + w],
                                vg[h][kb // 4][:, kb % 4, 0:65],
                                p[:, off : off + w],
                                start=(kb == 0),
                                stop=(kb == 4 * g + 3),
                            )
                    # normalize per group: oTs[pr][g][hr:hr+64] = oT[:64]/oT[64]
                    for g in range(NG):
                        rec = smallpool.tile([1, 512], F32, tag="rec", name="rec", bufs=2)
                        nc.vector.reciprocal(rec[:], oT[g][64:65, :])
                        recb = smallpool.tile([1, 512], BF16, tag="recb", name="recb", bufs=2)
                        nc.vector.tensor_copy(recb[:], rec[:])
                        bc = stp.tile([64, 512], F32, tag="bc", name="bc", bufs=1)
                        nc.tensor.matmul(bc[:], ones1[:], recb[:], start=True, stop=True)
                        bcs = smallpool.tile([64, 512], F32, tag="bcs", name="bcs", bufs=2)
                        nc.scalar.activation(bcs[:], bc[:], AF.Copy)
                        nc.vector.tensor_mul(
                            oTs[pr][g][hr : hr + 64, :],
                            oT[g][0:64, :],
                            bcs[:],
                        )

            # ================= phase 4: output projection =================
            with tc.tile_pool(name="outpsum", bufs=3, space=bass.MemorySpace.PSUM) as op:
                for qb in range(NB if _KP >= 5 else 0):
                    g, off = qb // 4, (qb % 4) * 128
                    po = op.tile([128, DOUT], F32, tag="po", name="po")
                    for pr in range(2):
                        for nh in range(2):
                            nc.tensor.matmul(
                                po[:, nh * 512 : (nh + 1) * 512],
                                oTs[pr][g][:, off : off + 128],
                                wo_sb[pr][:, nh * 512 : (nh + 1) * 512],
                                start=(pr == 0),
                                stop=(pr == 1),
                            )
                    ob = outsb_pool.tile([128, DOUT], BF16, tag="ob", name="ob")
                    if qb % 2 == 0:
                        nc.scalar.activation(ob[:], po[:], AF.Copy)
                    else:
                        nc.vector.tensor_copy(ob[:], po[:])
                    nc.sync.dma_start(out_ext[qb * 128 : (qb + 1) * 128, :], ob[:])

    nc.compile()
    return nc


def _prep_host(s_q, s_kv, mask_q, mask_kv, Wq, bq_, Wkv, bkv_, Wo, bo_):
    """Build per-core input maps (host-side shard + transform)."""
    perm = _head_perm()

    inv_freq = 1.0 / (10000.0 ** (np.arange(0, ROT, 2, dtype=np.float64) / ROT))
    t = np.arange(N, dtype=np.float64)[None, :] * inv_freq[:, None]  # [16, N]
    cosT = np.zeros((128, N), NPBF16)
    sinT = np.zeros((128, N), NPBF16)
    for rb in range(0, 128, 32):
        cosT[rb : rb + 16] = np.cos(t).astype(NPBF16)
        sgn = 1.0 if (rb // 32) % 2 == 0 else -1.0
        sinT[rb : rb + 16] = (sgn * np.sin(t)).astype(NPBF16)

    mt = np.zeros((128, 128), np.float32)
    pidx = np.arange(128)
    mt[pidx[:, None] > pidx[None, :]] = -INF
    mt = mt.astype(NPBF16)
    ident = np.eye(128, dtype=NPBF16)

    in_maps = []
    for core in range(8):
        b = core // 4
        h0 = (core % 4) * HPC

        wq_cols = []
        bq_cols = []
        for h in range(h0, h0 + HPC):
            cols = Wq[:, h * DH : (h + 1) * DH][:, perm]
            wq_cols.append(cols)
            bq_cols.append(bq_[h * DH : (h + 1) * DH][perm])
        wq_c = np.concatenate(wq_cols, axis=1)  # [1024, 256]
        bq_c = np.concatenate(bq_cols)  # [256]

        wkv_cols = []
        bkv_cols = []
        for h in range(h0, h0 + HPC):
            kcols = Wkv[:, h * 2 * DH : h * 2 * DH + DH][:, perm]
            vcols = Wkv[:, h * 2 * DH + DH : (h + 1) * 2 * DH]
            kb_ = bkv_[h * 2 * DH : h * 2 * DH + DH][perm]
            vb_ = bkv_[h * 2 * DH + DH : (h + 1) * 2 * DH]
            if (h - h0) % 2 == 0:  # even head: [k; v]
                wkv_cols.append(np.concatenate([kcols, vcols], axis=1))
                bkv_cols.append(np.concatenate([kb_, vb_]))
            else:  # odd head: [v; k] so k-rows sit at partition base 64
                wkv_cols.append(np.concatenate([vcols, kcols], axis=1))
                bkv_cols.append(np.concatenate([vb_, kb_]))
        wkv_c = np.concatenate(wkv_cols, axis=1)  # [1024, 512]

        wo_rows = Wo[h0 * DH : (h0 + HPC) * DH, :]  # [256, 1024]

        bmask = (INF * (mask_kv[b].astype(np.float32) - 1.0)).reshape(NB, 128, 1)

        in_maps.append(
            {
                "sqt": np.ascontiguousarray(s_q[b].T).astype(NPBF16),
                "skvt": np.ascontiguousarray(s_kv[b].T).astype(NPBF16),
                "wq": np.ascontiguousarray(wq_c.reshape(8, 128, HPC * DH)).astype(NPBF16),
                "wkv": np.ascontiguousarray(wkv_c.reshape(8, 128, HPC * 2 * DH)).astype(NPBF16),
                "wo": np.ascontiguousarray(wo_rows.reshape(2, 128, DOUT)).astype(NPBF16),
                "bq": bq_c.reshape(2, 128, 1).astype(np.float32),
                "bkv": np.stack(bkv_cols).reshape(HPC, 128, 1).astype(np.float32),
                "cost": cosT,
                "sint": sinT,
                "mtile": mt,
                "ident": ident,
                "bmask": bmask.astype(np.float32),
            }
        )
    return in_maps


_NC_CACHE = {}


def kernel(s_q, s_kv, mask_q, mask_kv, Wq, bq, Wkv, bkv, Wo, bo, _return_results=False):
    from concourse.bass_utils import run_bass_kernel_spmd

    if "nc" not in _NC_CACHE:
        _NC_CACHE["nc"] = build_nc()
    nc = _NC_CACHE["nc"]

    in_maps = _prep_host(
        np.asarray(s_q, np.float32),
        np.asarray(s_kv, np.float32),
        np.asarray(mask_q, np.float32),
        np.asarray(mask_kv, np.float32),
        np.asarray(Wq, np.float32),
        np.asarray(bq, np.float32),
        np.asarray(Wkv, np.float32),
        np.asarray(bkv, np.float32),
        np.asarray(Wo, np.float32),
        np.asarray(bo, np.float32),
    )
    trace = bool(int(os.environ.get("KERNEL_TRACE", "0")))
    res = run_bass_kernel_spmd(nc, in_maps, core_ids=list(range(8)), trace=trace)

    out = np.zeros((B, N, DOUT), np.float32)
    for core in range(8):
        b = core // 4
        out[b] += res.results[core]["out"].astype(np.float32)
    out += np.asarray(bo, np.float32)[None, None, :]
    if _return_results:
        return out, res
    return out



# revision 7
# speedup vs baseline: 1.2035x; 1.2035x over previous
"""Distributed Trainium2 Bass kernel for nn_AttentionLayer_25993142075512.

Sharding: 8 cores = 2 batches x 4 head-groups (4 heads each). Each core
computes its batch's q/k/v projections for its 4 heads, causal attention,
and a partial output projection o @ Wo[head_rows]. Host sums the 4
partials per batch and adds bo. No on-device collectives.

v2 design notes (vs baseline):
  - Fused phase pipeline: projections stream per 512-col sub; attention
    runs g-outer (q-group) with v'-build, scores, exp, o-accum, softmax
    normalization and the output projection all interleaved per group, so
    the PE never idles and stays at max p-state clock.
  - Causal diagonal mask applied on the PE: an extra accumulate matmul
    (-2000*I) @ tri into the score PSUM (start=False) replaces per-block
    DVE adds.
  - exp is split: diagonal tiles (and 1/4 of off-diag) use Scalar ACT
    exp; the rest use a one-op DVE Schraudolph exp -> int16 whose bits
    ARE the bf16 probs (bitcast feeds the o-matmul directly).
  - RoPE: head dims permuted host-side to [evens|odds|pass]; the
    rotate-partner is built by a PE permutation matmul, then 3 full-width
    DVE ops (t=shuf*sin, x*=cos, x+=t) per tile-sub.
  - Softmax denominators: ones column in v' (row 64 of oT). Reciprocal
    via one batched DVE reciprocal_approx_fast at partitions {0,32,64,96},
    broadcast via tiny ones-row matmuls.
  - All attention-phase PSUM lives in one 4-bank oT tile + one 4-slot
    ring shared by score tiles / v'-transposes / norm-broadcast / out-proj.
"""

import sys, os, types, ctypes, contextlib

sys.path.insert(0, "/opt/trn_rl_repo")

import numpy as np
import ml_dtypes


def _install_axon_hooks():
    so = "/opt/axon/libaxon_pjrt.so"

    def _hook_factory(so_path):
        if not os.path.exists(so_path):
            return None
        lib = ctypes.CDLL(so_path)
        if not hasattr(lib, "axon_start_nrt_profile"):
            return None
        lib.axon_start_nrt_profile.argtypes = [
            ctypes.POINTER(ctypes.c_int64),
            ctypes.c_size_t,
        ]
        lib.axon_start_nrt_profile.restype = ctypes.c_int64
        lib.axon_stop_nrt_profile.argtypes = [ctypes.c_char_p]
        lib.axon_stop_nrt_profile.restype = ctypes.c_int64

        @contextlib.contextmanager
        def _hook(output_dir, device_ids):
            import jax

            jax.devices()
            if device_ids:
                ids = (ctypes.c_int64 * len(device_ids))(*device_ids)
                rc = lib.axon_start_nrt_profile(ids, len(device_ids))
            else:
                rc = lib.axon_start_nrt_profile(None, 0)
            if rc != 0:
                raise RuntimeError(f"axon_start_nrt_profile rc={rc}")
            try:
                yield
            finally:
                n = lib.axon_stop_nrt_profile(str(output_dir).encode())
                if n < 0:
                    raise RuntimeError(f"axon_stop_nrt_profile rc={n}")

        return _hook

    try:
        import antenv

        if "antenv.axon_hooks" not in sys.modules:
            hook = _hook_factory(so)
            mod = types.ModuleType("antenv.axon_hooks")
            mod.get_axon_ntff_profile_hook = lambda: hook
            mod.set_axon_ntff_profile_hook = lambda h: None
            antenv.axon_hooks = mod
            sys.modules["antenv.axon_hooks"] = mod
    except ImportError:
        pass
    from concourse import bass_utils

    bass_utils.upload_artifacts = lambda tmpdir: tmpdir


_install_axon_hooks()

from concourse import bass, bacc, tile, mybir  # noqa: E402

BF16 = mybir.dt.bfloat16
F32 = mybir.dt.float32
I16 = mybir.dt.int16
NPBF16 = ml_dtypes.bfloat16

B, N, DQ, DKV, H, DH, DOUT = 2, 2048, 1024, 1024, 16, 64, 1024
ROT = DH // 2  # 32
HPC = 4  # heads per core
NB = N // 128  # 16 q/k blocks
NG = NB // 4  # 4 q-block groups (packs of 4)
NSUB = 4
SW = N // NSUB  # 512
VS = 72  # v' tile stride (64 v cols + ones col + pad)

TRI_C = -2000.0  # causal-mask additive constant (safe for int16 exp path)
SCH_A = 128.0 / np.log(2.0)  # Schraudolph bf16 scale
SCH_C = 8.5  # Schraudolph magic (calibrated)
MASK_BIAS = -30000.0  # scalar-path masked bias


def build_nc():
    nc = bacc.Bacc(None, target_bir_lowering=False)

    sqt = nc.declare_dram_parameter("sqt", [DQ, N], BF16, isOutput=False)
    skvt = nc.declare_dram_parameter("skvt", [DKV, N], BF16, isOutput=False)
    wq = nc.declare_dram_parameter("wq", [8, 128, HPC * DH], BF16, isOutput=False)
    wkv = nc.declare_dram_parameter("wkv", [8, 128, HPC * 2 * DH], BF16, isOutput=False)
    wo = nc.declare_dram_parameter("wo", [2, 128, DOUT], BF16, isOutput=False)
    bq = nc.declare_dram_parameter("bq", [2, 128, 1], F32, isOutput=False)
    bkv = nc.declare_dram_parameter("bkv", [HPC, 128, 1], F32, isOutput=False)
    cost_d = nc.declare_dram_parameter("cost", [128, N], BF16, isOutput=False)
    sint_d = nc.declare_dram_parameter("sint", [128, N], BF16, isOutput=False)
    perm_d = nc.declare_dram_parameter("permm", [128, 128], BF16, isOutput=False)
    negi_d = nc.declare_dram_parameter("negi", [128, 128], BF16, isOutput=False)
    tri_d = nc.declare_dram_parameter("tri", [128, 128], BF16, isOutput=False)
    ident_d = nc.declare_dram_parameter("ident", [128, 128], BF16, isOutput=False)
    bmask_d = nc.declare_dram_parameter("bmask", [NB, 128, 1], F32, isOutput=False)
    b2_d = nc.declare_dram_parameter("b2", [NB, 128, 1], F32, isOutput=False)
    out_ext = nc.declare_dram_parameter("out", [N, DOUT], BF16, isOutput=True)

    AF = mybir.ActivationFunctionType
    ALU = mybir.AluOpType

    with tile.TileContext(nc) as tc:
        with (
            tc.tile_pool(name="const", bufs=1) as cpool,
            tc.tile_pool(name="big", bufs=1) as bigpool,
            tc.tile_pool(name="stream", bufs=1) as spool,
            tc.tile_pool(name="ptile", bufs=1) as ppool,
            tc.tile_pool(name="small", bufs=1) as smallpool,
        ):
            # ---- constant tiles ----
            wq_sb = [cpool.tile([128, HPC * DH], BF16, tag=f"wq{c}", name=f"wq{c}") for c in range(8)]
            wkv_sb = [cpool.tile([128, HPC * 2 * DH], BF16, tag=f"wkv{c}", name=f"wkv{c}") for c in range(8)]
            wo_sb = [cpool.tile([128, DOUT], BF16, tag=f"wo{pr}", name=f"wo{pr}") for pr in range(2)]
            bq_sb = cpool.tile([128, 2], F32, tag="bq", name="bq")
            bkv_sb = cpool.tile([128, HPC], F32, tag="bkv", name="bkv")
            cost = cpool.tile([128, N], BF16, tag="cost", name="cost")
            sint = cpool.tile([128, N], BF16, tag="sint", name="sint")
            permm = cpool.tile([128, 128], BF16, tag="permm", name="permm")
            negi = cpool.tile([128, 128], BF16, tag="negi", name="negi")
            tri = cpool.tile([128, 128], BF16, tag="tri", name="tri")
            ident = cpool.tile([128, 128], BF16, tag="ident", name="ident")
            bmask = cpool.tile([128, NB], F32, tag="bmask", name="bmask")
            b2 = cpool.tile([128, NB], F32, tag="b2", name="b2")
            ones1 = cpool.tile([33, 64], BF16, tag="ones1", name="ones1")
            dn = cpool.tile([33, 2, 512], F32, tag="dn", name="dn")

            # ---- persistent activations ----
            qT = [bigpool.tile([128, N], BF16, tag=f"qT{i}", name=f"qT{i}") for i in range(2)]
            kvT = [bigpool.tile([128, N], BF16, tag=f"kvT{h}", name=f"kvT{h}") for h in range(HPC)]
            vg = [
                [bigpool.tile([128, 4, VS], BF16, tag=f"vg{h}_{g}", name=f"vg{h}_{g}") for g in range(NG)]
                for h in range(HPC)
            ]
            oTs = [
                [bigpool.tile([128, 512], BF16, tag=f"oTs{pr}_{g}", name=f"oTs{pr}_{g}") for g in range(NG)]
                for pr in range(2)
            ]

            # early DMAs: q-projection weights + biases first, then sub-0 xq
            nc.sync.dma_start(bq_sb[:, 0:1], bq[0])
            nc.sync.dma_start(bq_sb[:, 1:2], bq[1])
            for h in range(HPC):
                nc.sync.dma_start(bkv_sb[:, h : h + 1], bkv[h])
            for c in range(8):
                nc.sync.dma_start(wq_sb[c][:], wq[c])

            xqs = [[None] * 8 for _ in range(NSUB)]
            xkvs = [[None] * 8 for _ in range(NSUB)]

            def dma_x(sub, kv):
                s0 = sub * SW
                for c in range(8):
                    t = spool.tile(
                        [128, SW], BF16, tag="xkv" if kv else "xq", name="x", bufs=16
                    )
                    src = skvt if kv else sqt
                    nc.sync.dma_start(t[:], src[c * 128 : (c + 1) * 128, s0 : s0 + SW])
                    if kv:
                        xkvs[sub][c] = t
                    else:
                        xqs[sub][c] = t

            dma_x(0, False)
            # rope constants next (needed right after sub-0 q projection)
            nc.sync.dma_start(permm[:], perm_d[:])
            nc.sync.dma_start(cost[:], cost_d[:])
            nc.sync.dma_start(sint[:], sint_d[:])
            for c in range(8):
                nc.sync.dma_start(wkv_sb[c][:], wkv[c])
            dma_x(0, True)
            # remaining constants (needed by attention, ~15us in)
            nc.sync.dma_start(negi[:], negi_d[:])
            nc.sync.dma_start(tri[:], tri_d[:])
            nc.sync.dma_start(ident[:], ident_d[:])
            for kb in range(NB):
                nc.sync.dma_start(bmask[:, kb : kb + 1], bmask_d[kb])
                nc.sync.dma_start(b2[:, kb : kb + 1], b2_d[kb])
            for pr in range(2):
                nc.sync.dma_start(wo_sb[pr][:], wo[pr])
            nc.vector.memset(ones1[:], 1.0)
            nc.vector.memset(dn[:], 1.0)
            for sub in range(1, NSUB):
                dma_x(sub, False)
                dma_x(sub, True)

            # ================= phase A: projections + rope =================
            with (
                tc.tile_pool(name="projpsum", bufs=1, space=bass.MemorySpace.PSUM) as pj,
                tc.tile_pool(name="shufpsum", bufs=1, space=bass.MemorySpace.PSUM) as shp,
            ):
                def rope_unit(dst, rbase, rows, cs):
                    """RoPE in place on dst[rbase:rbase+rows, cs] (rows 64 or 128).
                    Table rows [rbase:rbase+rows] hold the coefficients."""
                    sh = shp.tile([128, SW], F32, tag="sh", name="sh", bufs=2)
                    rsl = slice(rbase, rbase + rows)
                    nc.tensor.matmul(
                        sh[rsl, 0:SW],
                        permm[rsl, rbase : rbase + rows],
                        dst[rsl, cs],
                        start=True,
                        stop=True,
                    )
                    tsb = smallpool.tile([128, SW], BF16, tag="ropet", name="ropet", bufs=2)
                    tsc = smallpool.tile([128, SW], BF16, tag="ropec", name="ropec", bufs=2)
                    v = nc.vector
                    v.tensor_mul(tsb[rsl, :], sh[rsl, 0:SW], sint[rsl, cs])
                    v.tensor_mul(tsc[rsl, :], dst[rsl, cs], cost[rsl, cs])
                    v.tensor_add(dst[rsl, cs], tsc[rsl, :], tsb[rsl, :])

                for sub in range(NSUB):
                    nq0 = sub * SW
                    cs = slice(nq0, nq0 + SW)
                    # q chains
                    for m in range(2):
                        ps = pj.tile([128, SW], F32, tag="pj", name="pj", bufs=2)
                        for c in range(8):
                            nc.tensor.matmul(
                                ps[:],
                                wq_sb[c][:, m * 128 : (m + 1) * 128],
                                xqs[sub][c][:],
                                start=(c == 0),
                                stop=(c == 7),
                            )
                        nc.scalar.activation(
                            qT[m][:, cs], ps[:], AF.Identity, bias=bq_sb[:, m : m + 1]
                        )
                        rope_unit(qT[m], 0, 128, cs)
                    # kv chains
                    for h in range(HPC):
                        ps = pj.tile([128, SW], F32, tag="pj", name="pj", bufs=2)
                        for c in range(8):
                            nc.tensor.matmul(
                                ps[:],
                                wkv_sb[c][:, h * 128 : (h + 1) * 128],
                                xkvs[sub][c][:],
                                start=(c == 0),
                                stop=(c == 7),
                            )
                        nc.scalar.activation(
                            kvT[h][:, cs], ps[:], AF.Identity, bias=bkv_sb[:, h : h + 1]
                        )
                        rope_unit(kvT[h], (h % 2) * 64, 64, cs)

            # ================= phase C: attention + out-proj, g-outer =================
            with (
                tc.tile_pool(name="otpsum", bufs=1, space=bass.MemorySpace.PSUM) as otp,
                tc.tile_pool(name="ringp", bufs=1, space=bass.MemorySpace.PSUM) as ringp,
            ):
                # oT: [65, 4, 512] f32 = 4 banks; head h accumulates in bank h
                oT = otp.tile([65, HPC, 512], F32, tag="oT", name="oT")

                def ring(tag_w=512):
                    return ringp.tile([128, 512], F32, tag="ring", name="ring", bufs=4)

                exp_cnt = [0]
                for g in range(NG):
                    # ---- v' build for block-group g ----
                    for h in range(HPC):
                        vb = 64 if h % 2 == 0 else 0
                        nc.gpsimd.memset(vg[h][g][:], 1.0)
                        pk = ring()[:, 0:128].bitcast(BF16)  # [128, 256] bf16 view
                        for j in range(4):
                            kb = 4 * g + j
                            nc.tensor.matmul(
                                pk[:, j * 64 : (j + 1) * 64],
                                kvT[h][vb : vb + 64, kb * 128 : (kb + 1) * 128],
                                ident[vb : vb + 64, vb : vb + 64],
                                is_transpose=True,
                                start=(j == 0),
                                stop=(j == 3),
                            )
                        nc.scalar.activation(vg[h][g][:, :, 0:64], pk[:, 0:256], AF.Copy)
                    # ---- scores / exp / o-accumulate ----
                    for h in range(HPC):
                        kr = (h % 2) * 64
                        pr, hr = h // 2, (h % 2) * 64
                        for kb in range(4 * g + 4):
                            diag = kb // 4 == g
                            off = (kb % 4) * 128 if diag else 0
                            w = 512 - off
                            sT = ring()
                            nc.tensor.matmul(
                                sT[:, off : off + w],
                                kvT[h][kr : kr + 64, kb * 128 : (kb + 1) * 128],
                                qT[pr][hr : hr + 64, g * 512 + off : g * 512 + off + w],
                                start=True,
                                stop=not diag,
                            )
                            if diag:
                                nc.tensor.matmul(
                                    sT[:, off : off + 128],
                                    negi[:],
                                    tri[:],
                                    start=False,
                                    stop=True,
                                )
                            use_scalar = diag or (exp_cnt[0] % 4 == 0)
                            if not diag:
                                exp_cnt[0] += 1
                            if use_scalar:
                                p = ppool.tile([128, 512], BF16, tag="p", name="p", bufs=4)
                                nc.scalar.activation(
                                    p[:, off : off + w],
                                    sT[:, off : off + w],
                                    AF.Exp,
                                    bias=bmask[:, kb : kb + 1],
                                    scale=0.125,
                                )
                                pmv = p[:, off : off + w]
                            else:
                                pi = ppool.tile([128, 512], I16, tag="pi", name="pi", bufs=4)
                                nc.vector.tensor_scalar(
                                    pi[:, off : off + w],
                                    sT[:, off : off + w],
                                    0.125 * SCH_A,
                                    b2[:, kb : kb + 1],
                                    ALU.mult,
                                    ALU.add,
                                )
                                pmv = pi[:, off : off + w].bitcast(BF16)
                            nc.tensor.matmul(
                                oT[:, h, off : off + w],
                                vg[h][kb // 4][:, kb % 4, 0:65],
                                pmv,
                                start=(kb == 0),
                                stop=(kb == 4 * g + 3),
                            )
                    # ---- normalization for this g ----
                    rec = smallpool.tile([33, 2, 512], F32, tag="rec", name="rec", bufs=2)
                    recb = smallpool.tile([33, 2, 512], BF16, tag="recb", name="recb", bufs=2)
                    for h in range(HPC):
                        r0 = 32 * (h // 2)
                        nc.vector.tensor_copy(
                            dn[r0 : r0 + 1, h % 2, :], oT[64:65, h, :]
                        )
                    nc.vector.reciprocal_approx_fast(rec[:], dn[:])
                    nc.vector.tensor_copy(recb[:], rec[:])
                    for h in range(HPC):
                        pr, hr = h // 2, (h % 2) * 64
                        r0 = 32 * (h // 2)
                        bc = ring()
                        nc.tensor.matmul(
                            bc[0:64, :],
                            ones1[r0 : r0 + 1, :],
                            recb[r0 : r0 + 1, h % 2, :],
                            start=True,
                            stop=True,
                        )
                        bcs = smallpool.tile([64, 512], F32, tag="bcs", name="bcs", bufs=2)
                        nc.scalar.activation(bcs[:], bc[0:64, :], AF.Copy)
                        nc.vector.tensor_mul(
                            oTs[pr][g][hr : hr + 64, :], oT[0:64, h, :], bcs[:]
                        )
                    # ---- output projection for this g ----
                    for qb in range(4 * g, 4 * g + 4):
                        off = (qb % 4) * 128
                        for nh in range(2):
                            po = ring()
                            for pr in range(2):
                                nc.tensor.matmul(
                                    po[:],
                                    oTs[pr][g][:, off : off + 128],
                                    wo_sb[pr][:, nh * 512 : (nh + 1) * 512],
                                    start=(pr == 0),
                                    stop=(pr == 1),
                                )
                            ob = smallpool.tile([128, 512], BF16, tag="ob", name="ob", bufs=4)
                            if (qb + nh) % 2 == 0:
                                nc.scalar.activation(ob[:], po[:], AF.Copy)
                            else:
                                nc.vector.tensor_copy(ob[:], po[:])
                            nc.sync.dma_start(
                                out_ext[qb * 128 : (qb + 1) * 128, nh * 512 : (nh + 1) * 512],
                                ob[:],
                            )

    nc.compile()
    return nc


def _head_perm():
    """Permute one head's 64 dims: [evens(0,2..30), odds(1,3..31), pass 32:64]."""
    ev = np.arange(0, ROT, 2)
    od = np.arange(1, ROT, 2)
    return np.concatenate([ev, od, np.arange(ROT, DH)])


def _prep_host(s_q, s_kv, mask_q, mask_kv, Wq, bq_, Wkv, bkv_, Wo, bo_):
    """Build per-core input maps (host-side shard + transform)."""
    perm = _head_perm()

    # RoPE tables [128, N]: per 64-row block: rows 0:16 evens (cos, +sin),
    # rows 16:32 odds (cos, -sin), rows 32:64 pass (1, 0).
    inv_freq = 1.0 / (10000.0 ** (np.arange(0, ROT, 2, dtype=np.float64) / ROT))
    t = np.arange(N, dtype=np.float64)[None, :] * inv_freq[:, None]  # [16, N]
    cosT = np.zeros((128, N), np.float32)
    sinT = np.zeros((128, N), np.float32)
    cosT[:, :] = 1.0
    for rb in (0, 64):
        cosT[rb : rb + 16] = np.cos(t)
        cosT[rb + 16 : rb + 32] = np.cos(t)
        sinT[rb : rb + 16] = -np.sin(t)
        sinT[rb + 16 : rb + 32] = np.sin(t)
    cosT = cosT.astype(NPBF16)
    sinT = sinT.astype(NPBF16)

    # partner permutation matrix: permm[r, p] = 1 iff r = partner(p)
    pm = np.zeros((128, 128), np.float32)
    for p in range(128):
        b = p % 64
        if b < 16:
            partner = p + 16
        elif b < 32:
            partner = p - 16
        else:
            partner = p
        pm[partner, p] = 1.0
    pm = pm.astype(NPBF16)

    negi = (TRI_C * np.eye(128, dtype=np.float32)).astype(NPBF16)
    pidx = np.arange(128)
    trim = (pidx[:, None] > pidx[None, :]).astype(np.float32).astype(NPBF16)
    ident = np.eye(128, dtype=NPBF16)

    in_maps = []
    for core in range(8):
        b = core // 4
        h0 = (core % 4) * HPC

        wq_cols = []
        bq_cols = []
        for h in range(h0, h0 + HPC):
            wq_cols.append(Wq[:, h * DH : (h + 1) * DH][:, perm])
            bq_cols.append(bq_[h * DH : (h + 1) * DH][perm])
        wq_c = np.concatenate(wq_cols, axis=1)  # [1024, 256]
        bq_c = np.concatenate(bq_cols)  # [256]

        wkv_cols = []
        bkv_cols = []
        for h in range(h0, h0 + HPC):
            kcols = Wkv[:, h * 2 * DH : h * 2 * DH + DH][:, perm]
            vcols = Wkv[:, h * 2 * DH + DH : (h + 1) * 2 * DH]
            kb_ = bkv_[h * 2 * DH : h * 2 * DH + DH][perm]
            vb_ = bkv_[h * 2 * DH + DH : (h + 1) * 2 * DH]
            if (h - h0) % 2 == 0:  # even head: [k; v]
                wkv_cols.append(np.concatenate([kcols, vcols], axis=1))
                bkv_cols.append(np.concatenate([kb_, vb_]))
            else:  # odd head: [v; k] so k-rows sit at partition base 64
                wkv_cols.append(np.concatenate([vcols, kcols], axis=1))
                bkv_cols.append(np.concatenate([vb_, kb_]))
        wkv_c = np.concatenate(wkv_cols, axis=1)  # [1024, 512]

        wo_rows = Wo[h0 * DH : (h0 + HPC) * DH, :]  # [256, 1024]

        braw = 1.0e6 * (mask_kv[b].astype(np.float64) - 1.0)  # 0 or -1e6
        bmask = np.clip(braw, MASK_BIAS, 0.0).reshape(NB, 128, 1)
        b2v = (
            np.clip(SCH_A * braw, -40000.0, 0.0) + (127.0 * 128.0 - SCH_C)
        ).reshape(NB, 128, 1)

        in_maps.append(
            {
                "sqt": np.ascontiguousarray(s_q[b].T).astype(NPBF16),
                "skvt": np.ascontiguousarray(s_kv[b].T).astype(NPBF16),
                "wq": np.ascontiguousarray(wq_c.reshape(8, 128, HPC * DH)).astype(NPBF16),
                "wkv": np.ascontiguousarray(wkv_c.reshape(8, 128, HPC * 2 * DH)).astype(NPBF16),
                "wo": np.ascontiguousarray(wo_rows.reshape(2, 128, DOUT)).astype(NPBF16),
                "bq": bq_c.reshape(2, 128, 1).astype(np.float32),
                "bkv": np.stack(bkv_cols).reshape(HPC, 128, 1).astype(np.float32),
                "cost": cosT,
                "sint": sinT,
                "permm": pm,
                "negi": negi,
                "tri": trim,
                "ident": ident,
                "bmask": bmask.astype(np.float32),
                "b2": b2v.astype(np.float32),
            }
        )
    return in_maps


_NC_CACHE = {}


def kernel(s_q, s_kv, mask_q, mask_kv, Wq, bq, Wkv, bkv, Wo, bo, _return_results=False):
    from concourse.bass_utils import run_bass_kernel_spmd

    if "nc" not in _NC_CACHE:
        _NC_CACHE["nc"] = build_nc()
    nc = _NC_CACHE["nc"]

    in_maps = _prep_host(
        np.asarray(s_q, np.float32),
        np.asarray(s_kv, np.float32),
        np.asarray(mask_q, np.float32),
        np.asarray(mask_kv, np.float32),
        np.asarray(Wq, np.float32),
        np.asarray(bq, np.float32),
        np.asarray(Wkv, np.float32),
        np.asarray(bkv, np.float32),
        np.asarray(Wo, np.float32),
        np.asarray(bo, np.float32),
    )
    trace = bool(int(os.environ.get("KERNEL_TRACE", "0")))
    res = run_bass_kernel_spmd(nc, in_maps, core_ids=list(range(8)), trace=trace)

    out = np.zeros((B, N, DOUT), np.float32)
    for core in range(8):
        b = core // 4
        out[b] += res.results[core]["out"].astype(np.float32)
    out += np.asarray(bo, np.float32)[None, None, :]
    if _return_results:
        return out, res
    return out
